# revision 27
# baseline (speedup 1.0000x reference)
"""Trainium2 Bass kernel for nn_CrossAttentionReranker — feature-major, software-pipelined.

Reference math (seq_len==1) collapses MHA(x_q, x_kv) to x_kv @ wa with
wa = wv.T @ out_w.T folded on host; ln_w==1, ln_b==0, all biases 0 (asserted).

Layout: activations live TRANSPOSED ("feature-major"): features on the 128
partitions (4 chunks of 128 for D=512), rows on the free dim (R=512 rows per
macrotile). Matmuls need no PE transposes: for y = x @ W,
yT[mc] = sum_kc W[kc,mc].T @ xT[kc] with the weight chunk stationary.  The
candidate input is pre-transposed on the host.

LN (over the partition dim): residuals ride the matmul accumulation as
identity matmuls (layer-0 q residual q0 is per-partition and rides the ScalarE
evac bias); S1/S2 row sums via ones-column matmuls; mu/rstd rows broadcast
across partitions on GpSimd (partition_broadcast); apply via two
tensor_tensor ops (2x bf16 mode).  Squares for S2 run on GpSimd.  ScalarE uses
only Copy/Identity/Relu/Sqrt (one table set); one final Sigmoid.

The emission order is software-pipelined: stages are interleaved across G=2
macrotiles so each engine's in-order stream always has independent work from
the sibling macrotile while the other's LN chain is in flight.
"""

import sys

import numpy as np
import ml_dtypes

N = 131072
D = 512
HID = 256
L = 2
P = 128
R = 512          # rows per macrotile (free dim)
G = 4            # macrotiles in flight (software pipelining)
NCORES = 8
EPS = 1e-5

BF16 = ml_dtypes.bfloat16

_cache: dict = {}


def _chunk_lhsT(w: np.ndarray) -> np.ndarray:
    """[K, M] -> [128, (K//128)*(M//128)*128]; block (kc, mc) at col
    (kc*nmc + mc)*128, element (kp, mp) at [kp, block*128 + mp]."""
    k, m = w.shape
    nkc, nmc = k // P, m // P
    return np.ascontiguousarray(
        w.reshape(nkc, P, nmc, P).transpose(1, 0, 2, 3).reshape(P, nkc * nmc * P)
    )


def _prep_host(inputs):
    """Fold weights on host (fp64), cast bf16, chunk for lhsT layout."""
    f8 = np.float64
    assert np.all(np.asarray(inputs["ln_w"]) == 1.0), "kernel assumes ln_w == 1"
    assert not np.any(np.asarray(inputs["ln_b"])), "kernel assumes ln_b == 0"
    for k in ("attn_in_b", "attn_out_b", "ffn_b1", "ffn_b2", "head_b1", "head_b2"):
        assert not np.any(np.asarray(inputs[k])), f"kernel assumes {k} == 0"

    arrs = {}
    for i in range(L):
        wv = np.asarray(inputs["attn_in_w"])[i][2 * D:].astype(f8)   # [D, D]
        ow = np.asarray(inputs["attn_out_w"])[i].astype(f8)          # [D, D]
        wa = wv.T @ ow.T                                             # y = x @ wa
        arrs[f"wa{i}"] = _chunk_lhsT(wa).astype(BF16)                # [128, 16*128]
        w1 = np.asarray(inputs["ffn_w1"])[i].T.astype(f8)            # [512, 256]
        arrs[f"w1_{i}"] = _chunk_lhsT(w1).astype(BF16)               # [128, 8*128]
        w2 = np.asarray(inputs["ffn_w2"])[i].T.astype(f8)            # [256, 512]
        arrs[f"w2_{i}"] = _chunk_lhsT(w2).astype(BF16)               # [128, 8*128]
        arrs[f"csw2_{i}"] = np.ascontiguousarray(
            w2.sum(axis=1).reshape(2, P).T
        ).astype(BF16)                                               # [128, 2]
    arrs["h1"] = _chunk_lhsT(np.asarray(inputs["head_w1"]).T.astype(f8)).astype(BF16)
    arrs["h2"] = np.ascontiguousarray(
        np.asarray(inputs["head_w2"]).T.astype(f8).reshape(2, P).T
    ).astype(BF16)                                                   # [128, 2]
    arrs["q0T"] = np.ascontiguousarray(
        np.asarray(inputs["query_embedding"]).astype(np.float32).reshape(4, P).T
    )                                                                # [128, 4] f32
    ones = np.ones((P, 1), np.float32)
    arrs["onesc"] = ones.astype(BF16)                                # [128, 1]
    arrs["onesr"] = np.ones((1, P), np.float32).astype(BF16)         # [1, 128]
    arrs["ident"] = np.eye(P, dtype=np.float32).astype(BF16)
    return arrs


def _cand_T_for_core(cand_bf16: np.ndarray) -> np.ndarray:
    """[rows, 512] bf16 -> [NT*128, 4*R]: macrotile t at rows [t*128,(t+1)*128),
    chunk c at cols [c*R,(c+1)*R) — one contiguous 512KB DRAM block per
    macrotile load."""
    rows = cand_bf16.shape[0]
    nt = rows // R
    t4 = cand_bf16.T.reshape(4, P, nt, R)          # [c, p, t, r]
    return np.ascontiguousarray(
        t4.transpose(2, 1, 0, 3).reshape(nt * P, 4 * R)
    )


def _build_program(rows_per_core: int):
    import concourse.bass as bass
    import concourse.mybir as mybir
    import concourse.tile as tile
    from concourse import bacc
    from concourse.bass import ts

    dt = mybir.dt
    alu = mybir.AluOpType
    act_fn = mybir.ActivationFunctionType
    NT = rows_per_core // R
    assert rows_per_core % R == 0 and NT % G == 0

    nc = bacc.Bacc("TRN2", target_bir_lowering=False, debug=False,
                   num_devices=NCORES)

    candT = nc.dram_tensor("candT", [(rows_per_core // R) * P, 4 * R],
                           dt.bfloat16, kind="ExternalInput")
    dr = {}
    for i in range(L):
        dr[f"wa{i}"] = nc.dram_tensor(f"wa{i}", [P, 16 * P], dt.bfloat16, kind="ExternalInput")
        dr[f"w1_{i}"] = nc.dram_tensor(f"w1_{i}", [P, 8 * P], dt.bfloat16, kind="ExternalInput")
        dr[f"w2_{i}"] = nc.dram_tensor(f"w2_{i}", [P, 8 * P], dt.bfloat16, kind="ExternalInput")
        dr[f"csw2_{i}"] = nc.dram_tensor(f"csw2_{i}", [P, 2], dt.bfloat16, kind="ExternalInput")
    dr["h1"] = nc.dram_tensor("h1", [P, 16 * P], dt.bfloat16, kind="ExternalInput")
    dr["h2"] = nc.dram_tensor("h2", [P, 2], dt.bfloat16, kind="ExternalInput")
    dr["q0T"] = nc.dram_tensor("q0T", [P, 4], dt.float32, kind="ExternalInput")
    dr["onesc"] = nc.dram_tensor("onesc", [P, 1], dt.bfloat16, kind="ExternalInput")
    dr["onesr"] = nc.dram_tensor("onesr", [1, P], dt.bfloat16, kind="ExternalInput")
    dr["ident"] = nc.dram_tensor("ident", [P, P], dt.bfloat16, kind="ExternalInput")
    scores = nc.dram_tensor("scores", [rows_per_core, 1], dt.float32,
                            kind="ExternalOutput")

    from contextlib import ExitStack

    with tile.TileContext(nc) as tc, ExitStack() as ctx:
        const = ctx.enter_context(tc.tile_pool(name="const", bufs=1))

        def load_const(name, shape, dtype):
            t = const.tile(shape, dtype, tag=f"const_{name}")
            nc.sync.dma_start(t[:], dr[name].ap())
            return t

        wsb = []
        for i in range(L):
            wsb.append((load_const(f"wa{i}", [P, 16 * P], dt.bfloat16),
                        load_const(f"w1_{i}", [P, 8 * P], dt.bfloat16),
                        load_const(f"w2_{i}", [P, 8 * P], dt.bfloat16),
                        load_const(f"csw2_{i}", [P, 2], dt.bfloat16)))
        h1sb = load_const("h1", [P, 16 * P], dt.bfloat16)
        h2sb = load_const("h2", [P, 2], dt.bfloat16)
        q0sb = load_const("q0T", [P, 4], dt.float32)
        onesc = load_const("onesc", [P, 1], dt.bfloat16)
        onesr = load_const("onesr", [1, P], dt.bfloat16)
        ident = load_const("ident", [P, P], dt.bfloat16)

        eps_t = const.tile([1, 1], dt.float32, tag="eps")
        nc.gpsimd.memset(eps_t[:], float(EPS))
        logit_sb = const.tile([NT, R], dt.float32, tag="logits")

        cin = ctx.enter_context(tc.tile_pool(name="cin", bufs=4))
        zp = ctx.enter_context(tc.tile_pool(name="zp", bufs=7))
        sqp = ctx.enter_context(tc.tile_pool(name="sqp", bufs=4))
        up = ctx.enter_context(tc.tile_pool(name="up", bufs=3))
        apool = ctx.enter_context(tc.tile_pool(name="apool", bufs=12))
        rhp = ctx.enter_context(tc.tile_pool(name="rhp", bufs=5))
        bcp = ctx.enter_context(tc.tile_pool(name="bcp", bufs=5))
        smp = ctx.enter_context(tc.tile_pool(name="smp", bufs=2))
        pm = ctx.enter_context(tc.tile_pool(name="pm", bufs=3, space="PSUM"))
        pstat = ctx.enter_context(tc.tile_pool(name="pstat", bufs=2, space="PSUM"))

        def mm_block(W_sb, nkc, nmc, x, resid=None):
            """Paired outputs: (nmc // 2) PSUM tiles [128, 2R] (2 banks each)."""
            outs = []
            for pr in range(nmc // 2):
                ps = pm.tile([P, 2 * R], dt.float32, tag="mm")
                for half in range(2):
                    mc = pr * 2 + half
                    oap = ps[:, half * R:(half + 1) * R]
                    for kc in range(nkc):
                        nc.tensor.matmul(
                            oap, W_sb[:, ts(kc * nmc + mc, P)], x[:, ts(kc, R)],
                            start=(kc == 0),
                            stop=(kc == nkc - 1 and resid is None),
                        )
                    if resid is not None:
                        nc.tensor.matmul(oap, ident[:], resid[:, ts(mc, R)],
                                         start=False, stop=True)
                outs.append(ps)
            return outs

        def ln_a_multi(psls, q0_flag, s1_srcs, resid_dves):
            """Per-op interleaved across the G macrotiles."""
            n = len(psls)
            zs = [zp.tile([P, 4 * R], dt.bfloat16, name="z") for _ in range(n)]
            for pr in range(2):
                for g in range(n):
                    z, ps_list = zs[g], psls[g]
                    if q0_flag:
                        for half in range(2):
                            c = pr * 2 + half
                            nc.scalar.activation(
                                out=z[:, ts(c, R)],
                                in_=ps_list[pr][:, half * R:(half + 1) * R],
                                func=act_fn.Identity, bias=q0sb[:, c:c + 1])
                    elif resid_dves is not None:
                        sl = slice(pr * 2 * R, (pr + 1) * 2 * R)
                        nc.vector.tensor_tensor(out=z[:, sl], in0=ps_list[pr][:],
                                                in1=resid_dves[g][:, sl],
                                                op=alu.add)
                    else:
                        nc.scalar.copy(z[:, pr * 2 * R:(pr + 1) * 2 * R],
                                       ps_list[pr][:])
            s1ps = []
            for g in range(n):
                s1p = pstat.tile([1, R], dt.float32, tag="stat")
                if s1_srcs is not None:
                    rh, cs_sb = s1_srcs[g]
                    for kc in range(2):
                        nc.tensor.matmul(s1p[:, :], cs_sb[:, kc:kc + 1],
                                         rh[:, ts(kc, R)],
                                         start=(kc == 0), stop=(kc == 1))
                else:
                    for c in range(4):
                        nc.tensor.matmul(s1p[:, :], onesc[:], zs[g][:, ts(c, R)],
                                         start=(c == 0), stop=(c == 3))
                s1ps.append(s1p)
            sqs = [sqp.tile([P, 4 * R], dt.bfloat16, name="sq") for _ in range(n)]
            for g in range(n):
                nc.scalar.square(sqs[g][:, 0:2 * R], zs[g][:, 0:2 * R])
            for g in range(n):
                nc.vector.tensor_tensor(out=sqs[g][:, 2 * R:4 * R],
                                        in0=zs[g][:, 2 * R:4 * R],
                                        in1=zs[g][:, 2 * R:4 * R], op=alu.mult)
            s2ps = []
            for g in range(n):
                s2p = pstat.tile([1, R], dt.float32, tag="stat")
                for c in range(4):
                    nc.tensor.matmul(s2p[:, :], onesc[:], sqs[g][:, ts(c, R)],
                                     start=(c == 0), stop=(c == 3))
                s2ps.append(s2p)
            return [{"z": zs[g], "s1p": s1ps[g], "s2p": s2ps[g]}
                    for g in range(n)]

        def ln_b_multi(sts):
            n = len(sts)
            mus = [smp.tile([1, R], dt.bfloat16, tag="mu", name="mu") for _ in range(n)]
            for g in range(n):
                nc.scalar.mul(mus[g][:], sts[g]["s1p"][:], 1.0 / D)
            bpss = [pm.tile([P, 2 * R], dt.float32, tag="mm", name="bps") for _ in range(n)]
            for g in range(n):
                nc.tensor.matmul(bpss[g][:, 0:R], onesr[:], mus[g][:],
                                 start=True, stop=True)
            e2s = [smp.tile([1, R], dt.bfloat16, tag="e2", name="e2") for _ in range(n)]
            for g in range(n):
                nc.scalar.mul(e2s[g][:], sts[g]["s2p"][:], 1.0 / D)
            mu2s = [smp.tile([1, R], dt.bfloat16, tag="mu2", name="mu2") for _ in range(n)]
            for g in range(n):
                nc.vector.tensor_tensor(out=mu2s[g][:], in0=mus[g][:],
                                        in1=mus[g][:], op=alu.mult)
            vraws = [smp.tile([1, R], dt.bfloat16, tag="vraw", name="vraw") for _ in range(n)]
            for g in range(n):
                nc.vector.tensor_tensor(out=vraws[g][:], in0=e2s[g][:],
                                        in1=mu2s[g][:], op=alu.subtract)
            stds = [smp.tile([1, R], dt.float32, tag="std", name="std") for _ in range(n)]
            for g in range(n):
                nc.scalar.activation(out=stds[g][:], in_=vraws[g][:],
                                     func=act_fn.Sqrt, scale=1.0, bias=eps_t[:])
            rstdfs = [smp.tile([1, R], dt.float32, tag="rstdf", name="rstdf") for _ in range(n)]
            for g in range(n):
                nc.vector.reciprocal_approx_fast(out=rstdfs[g][:], in_=stds[g][:])
            rstds = [smp.tile([1, R], dt.bfloat16, tag="rstd", name="rstd") for _ in range(n)]
            for g in range(n):
                nc.scalar.copy(rstds[g][:], rstdfs[g][:])
            for g in range(n):
                nc.tensor.matmul(bpss[g][:, R:2 * R], onesr[:], rstds[g][:],
                                 start=True, stop=True)
            bcs = [bcp.tile([P, 2 * R], dt.bfloat16, tag="bc", name="bc") for _ in range(n)]
            for g in range(n):
                nc.scalar.copy(bcs[g][:], bpss[g][:])
            us = [up.tile([P, 4 * R], dt.bfloat16, name="u") for _ in range(n)]
            a_s = [apool.tile([P, 4 * R], dt.bfloat16, name="a") for _ in range(n)]
            for c in range(4):
                for g in range(n):
                    nc.vector.tensor_tensor(out=us[g][:, ts(c, R)],
                                            in0=sts[g]["z"][:, ts(c, R)],
                                            in1=bcs[g][:, 0:R], op=alu.subtract)
            for c in range(4):
                for g in range(n):
                    nc.vector.tensor_tensor(out=a_s[g][:, ts(c, R)],
                                            in0=us[g][:, ts(c, R)],
                                            in1=bcs[g][:, R:2 * R], op=alu.mult)
            return a_s

        def ffn_mm1(wsb_i, a_in):
            _, w1, _, _ = wsb_i
            hps = mm_block(w1, 4, 2, a_in)
            rh = rhp.tile([P, 2 * R], dt.bfloat16)
            nc.scalar.activation(out=rh[:], in_=hps[0][:], func=act_fn.Relu)
            return rh

        def head_mm(a2, a4):
            ps = pm.tile([P, 2 * R], dt.float32, tag="mm")
            for mc in range(2):
                oap = ps[:, mc * R:(mc + 1) * R]
                for kc in range(8):
                    x = a2 if kc < 4 else a4
                    nc.tensor.matmul(oap, h1sb[:, ts(kc * 2 + mc, P)],
                                     x[:, ts(kc % 4, R)],
                                     start=(kc == 0), stop=(kc == 7))
            rh = rhp.tile([P, 2 * R], dt.bfloat16)
            nc.scalar.activation(out=rh[:], in_=ps[:], func=act_fn.Relu)
            return rh

        def head_out(rh, t):
            lg = pstat.tile([1, R], dt.float32, tag="stat")
            for kc in range(2):
                nc.tensor.matmul(lg[:, :], h2sb[:, kc:kc + 1], rh[:, ts(kc, R)],
                                 start=(kc == 0), stop=(kc == 1))
            lgs = smp.tile([1, R], dt.float32, tag="lgs")
            nc.scalar.copy(lgs[:], lg[:])
            nc.sync.dma_start(logit_sb[t:t + 1, :], lgs[:])

        for t0 in range(0, NT, G):
            gs = list(range(G))
            tt = [t0 + g for g in gs]
            S = [dict() for _ in gs]

            for g in gs:
                cT = cin.tile([P, 4 * R], dt.bfloat16)
                nc.sync.dma_start(
                    cT[:], candT.ap()[tt[g] * P:(tt[g] + 1) * P, :])
                S[g]["c"] = cT
                S[g]["q"] = None

            for i in range(L):
                wa, w1, w2, csw2 = wsb[i]
                # q-stream attention
                for g in gs:
                    S[g]["ps"] = mm_block(wa, 4, 4, S[g]["c"])
                sts = ln_a_multi([S[g]["ps"] for g in gs], q0_flag=(i == 0),
                                 s1_srcs=None,
                                 resid_dves=None if i == 0 else
                                 [S[g]["q"] for g in gs])
                a1s = ln_b_multi(sts)
                # q-stream FFN
                rhs = [ffn_mm1(wsb[i], a1s[g]) for g in gs]
                for g in gs:
                    S[g]["ps"] = mm_block(w2, 2, 4, rhs[g])
                sts = ln_a_multi([S[g]["ps"] for g in gs], q0_flag=False,
                                 s1_srcs=[(rhs[g], csw2) for g in gs],
                                 resid_dves=a1s)
                a2s = ln_b_multi(sts)
                # c-stream attention
                for g in gs:
                    S[g]["ps"] = mm_block(wa, 4, 4, a2s[g])
                sts = ln_a_multi([S[g]["ps"] for g in gs], q0_flag=False,
                                 s1_srcs=None,
                                 resid_dves=[S[g]["c"] for g in gs])
                a3s = ln_b_multi(sts)
                # c-stream FFN
                rhs = [ffn_mm1(wsb[i], a3s[g]) for g in gs]
                for g in gs:
                    S[g]["ps"] = mm_block(w2, 2, 4, rhs[g])
                sts = ln_a_multi([S[g]["ps"] for g in gs], q0_flag=False,
                                 s1_srcs=[(rhs[g], csw2) for g in gs],
                                 resid_dves=a3s)
                a4s = ln_b_multi(sts)
                for g in gs:
                    S[g]["q"], S[g]["c"] = a2s[g], a4s[g]
                    S[g]["a2"], S[g]["a4"] = a2s[g], a4s[g]

            for g in gs:
                S[g]["hh"] = head_mm(S[g]["a2"], S[g]["a4"])
            for g in gs:
                head_out(S[g]["hh"], tt[g])

        fin = const.tile([NT, R], dt.float32, tag="fin")
        nc.scalar.activation(out=fin[:], in_=logit_sb[:], func=act_fn.Sigmoid)
        nc.sync.dma_start(scores.ap().rearrange("(t r) o -> t (r o)", r=R),
                          fin[:])

    nc.compile()
    return nc


def _get_program(rows_per_core: int):
    if rows_per_core not in _cache:
        _cache[rows_per_core] = _build_program(rows_per_core)
    return _cache[rows_per_core]


def _per_core_inputs(inputs) -> list:
    """Full inputs -> per-core input maps (shared weights + candT slice)."""
    arrs = _prep_host(inputs)
    cand = np.asarray(inputs["candidate_embeddings"]).astype(BF16)
    n = cand.shape[0]
    rows = n // NCORES
    in_maps = []
    for c in range(NCORES):
        m = dict(arrs)
        m["candT"] = _cand_T_for_core(cand[c * rows:(c + 1) * rows])
        in_maps.append(m)
    return in_maps


def kernel(**inputs) -> np.ndarray:
    from concourse.bass_utils import run_bass_kernel_spmd

    n = np.asarray(inputs["candidate_embeddings"]).shape[0]
    rows = n // NCORES
    nc = _get_program(rows)
    in_maps = _per_core_inputs(inputs)
    res = run_bass_kernel_spmd(nc, in_maps, list(range(NCORES)))
    out = np.concatenate([res.results[c]["scores"] for c in range(NCORES)], axis=0)
    return out.astype(np.float32)


if __name__ == "__main__":
    rows = int(sys.argv[1]) if len(sys.argv) > 1 else 4096
    nc = _build_program(rows)
    print("built ok:", rows)


# revision 30
# speedup vs baseline: 1.1406x; 1.1406x over previous
"""Trainium2 Bass kernel for nn_CrossAttentionReranker — feature-major, software-pipelined.

Reference math (seq_len==1) collapses MHA(x_q, x_kv) to x_kv @ wa with
wa = wv.T @ out_w.T folded on host; ln_w==1, ln_b==0, all biases 0 (asserted).

Layout: activations live TRANSPOSED ("feature-major"): features on the 128
partitions (4 chunks of 128 for D=512), rows on the free dim (R=512 rows per
macrotile). Matmuls need no PE transposes: for y = x @ W,
yT[mc] = sum_kc W[kc,mc].T @ xT[kc] with the weight chunk stationary.  The
candidate input is pre-transposed on the host.

LN (over the partition dim): residuals ride the matmul accumulation as
identity matmuls (layer-0 q residual q0 is per-partition and rides the ScalarE
evac bias); S1/S2 row sums via ones-column matmuls; mu/rstd rows broadcast
across partitions on GpSimd (partition_broadcast); apply via two
tensor_tensor ops (2x bf16 mode).  Squares for S2 run on GpSimd.  ScalarE uses
only Copy/Identity/Relu/Sqrt (one table set); one final Sigmoid.

The emission order is software-pipelined: stages are interleaved across G=2
macrotiles so each engine's in-order stream always has independent work from
the sibling macrotile while the other's LN chain is in flight.
"""

import sys

import numpy as np
import ml_dtypes

N = 131072
D = 512
HID = 256
L = 2
P = 128
R = 512          # rows per macrotile (free dim)
G = 4            # macrotiles in flight (software pipelining)
NCORES = 8
EPS = 1e-5

BF16 = ml_dtypes.bfloat16

_cache: dict = {}


def _chunk_lhsT(w: np.ndarray) -> np.ndarray:
    """[K, M] -> [128, (K//128)*(M//128)*128]; block (kc, mc) at col
    (kc*nmc + mc)*128, element (kp, mp) at [kp, block*128 + mp]."""
    k, m = w.shape
    nkc, nmc = k // P, m // P
    return np.ascontiguousarray(
        w.reshape(nkc, P, nmc, P).transpose(1, 0, 2, 3).reshape(P, nkc * nmc * P)
    )


def _prep_host(inputs):
    """Fold weights on host (fp64), cast bf16, chunk for lhsT layout."""
    f8 = np.float64
    assert np.all(np.asarray(inputs["ln_w"]) == 1.0), "kernel assumes ln_w == 1"
    assert not np.any(np.asarray(inputs["ln_b"])), "kernel assumes ln_b == 0"
    for k in ("attn_in_b", "attn_out_b", "ffn_b1", "ffn_b2", "head_b1", "head_b2"):
        assert not np.any(np.asarray(inputs[k])), f"kernel assumes {k} == 0"

    arrs = {}
    for i in range(L):
        wv = np.asarray(inputs["attn_in_w"])[i][2 * D:].astype(f8)   # [D, D]
        ow = np.asarray(inputs["attn_out_w"])[i].astype(f8)          # [D, D]
        wa = wv.T @ ow.T                                             # y = x @ wa
        arrs[f"wa{i}"] = _chunk_lhsT(wa).astype(BF16)                # [128, 16*128]
        w1 = np.asarray(inputs["ffn_w1"])[i].T.astype(f8)            # [512, 256]
        arrs[f"w1_{i}"] = _chunk_lhsT(w1).astype(BF16)               # [128, 8*128]
        w2 = np.asarray(inputs["ffn_w2"])[i].T.astype(f8)            # [256, 512]
        arrs[f"w2_{i}"] = _chunk_lhsT(w2).astype(BF16)               # [128, 8*128]
        arrs[f"csw2_{i}"] = np.ascontiguousarray(
            w2.sum(axis=1).reshape(2, P).T
        ).astype(BF16)                                               # [128, 2]
    arrs["h1"] = _chunk_lhsT(np.asarray(inputs["head_w1"]).T.astype(f8)).astype(BF16)
    arrs["h2"] = np.ascontiguousarray(
        np.asarray(inputs["head_w2"]).T.astype(f8).reshape(2, P).T
    ).astype(BF16)                                                   # [128, 2]
    arrs["q0T"] = np.ascontiguousarray(
        np.asarray(inputs["query_embedding"]).astype(np.float32).reshape(4, P).T
    )                                                                # [128, 4] f32
    ones = np.ones((P, 1), np.float32)
    arrs["onesc"] = ones.astype(BF16)                                # [128, 1]
    arrs["onesr"] = np.ones((1, P), np.float32).astype(BF16)         # [1, 128]
    arrs["ident"] = np.eye(P, dtype=np.float32).astype(BF16)
    return arrs


def _cand_T_for_core(cand_bf16: np.ndarray) -> np.ndarray:
    """[rows, 512] bf16 -> [NT*128, 4*R]: macrotile t at rows [t*128,(t+1)*128),
    chunk c at cols [c*R,(c+1)*R) — one contiguous 512KB DRAM block per
    macrotile load."""
    rows = cand_bf16.shape[0]
    nt = rows // R
    t4 = cand_bf16.T.reshape(4, P, nt, R)          # [c, p, t, r]
    return np.ascontiguousarray(
        t4.transpose(2, 1, 0, 3).reshape(nt * P, 4 * R)
    )


def _build_program(rows_per_core: int):
    import concourse.bass as bass
    import concourse.mybir as mybir
    import concourse.tile as tile
    from concourse import bacc
    from concourse.bass import ts

    dt = mybir.dt
    alu = mybir.AluOpType
    act_fn = mybir.ActivationFunctionType
    NT = rows_per_core // R
    assert rows_per_core % R == 0 and NT % G == 0

    nc = bacc.Bacc("TRN2", target_bir_lowering=False, debug=False,
                   num_devices=NCORES)

    candT = nc.dram_tensor("candT", [(rows_per_core // R) * P, 4 * R],
                           dt.bfloat16, kind="ExternalInput")
    dr = {}
    for i in range(L):
        dr[f"wa{i}"] = nc.dram_tensor(f"wa{i}", [P, 16 * P], dt.bfloat16, kind="ExternalInput")
        dr[f"w1_{i}"] = nc.dram_tensor(f"w1_{i}", [P, 8 * P], dt.bfloat16, kind="ExternalInput")
        dr[f"w2_{i}"] = nc.dram_tensor(f"w2_{i}", [P, 8 * P], dt.bfloat16, kind="ExternalInput")
        dr[f"csw2_{i}"] = nc.dram_tensor(f"csw2_{i}", [P, 2], dt.bfloat16, kind="ExternalInput")
    dr["h1"] = nc.dram_tensor("h1", [P, 16 * P], dt.bfloat16, kind="ExternalInput")
    dr["h2"] = nc.dram_tensor("h2", [P, 2], dt.bfloat16, kind="ExternalInput")
    dr["q0T"] = nc.dram_tensor("q0T", [P, 4], dt.float32, kind="ExternalInput")
    dr["onesc"] = nc.dram_tensor("onesc", [P, 1], dt.bfloat16, kind="ExternalInput")
    dr["onesr"] = nc.dram_tensor("onesr", [1, P], dt.bfloat16, kind="ExternalInput")
    dr["ident"] = nc.dram_tensor("ident", [P, P], dt.bfloat16, kind="ExternalInput")
    scores = nc.dram_tensor("scores", [rows_per_core, 1], dt.float32,
                            kind="ExternalOutput")

    from contextlib import ExitStack

    with tile.TileContext(nc) as tc, ExitStack() as ctx:
        const = ctx.enter_context(tc.tile_pool(name="const", bufs=1))

        def load_const(name, shape, dtype):
            t = const.tile(shape, dtype, tag=f"const_{name}")
            nc.sync.dma_start(t[:], dr[name].ap())
            return t

        wsb = []
        for i in range(L):
            wsb.append((load_const(f"wa{i}", [P, 16 * P], dt.bfloat16),
                        load_const(f"w1_{i}", [P, 8 * P], dt.bfloat16),
                        load_const(f"w2_{i}", [P, 8 * P], dt.bfloat16),
                        load_const(f"csw2_{i}", [P, 2], dt.bfloat16)))
        h1sb = load_const("h1", [P, 16 * P], dt.bfloat16)
        h2sb = load_const("h2", [P, 2], dt.bfloat16)
        q0sb = load_const("q0T", [P, 4], dt.float32)
        onesc = load_const("onesc", [P, 1], dt.bfloat16)
        onesr = load_const("onesr", [1, P], dt.bfloat16)
        ident = load_const("ident", [P, P], dt.bfloat16)

        eps_t = const.tile([1, 1], dt.float32, tag="eps")
        nc.gpsimd.memset(eps_t[:], float(EPS))
        logit_sb = const.tile([NT, R], dt.float32, tag="logits")

        cin = ctx.enter_context(tc.tile_pool(name="cin", bufs=4))
        zp = ctx.enter_context(tc.tile_pool(name="zp", bufs=7))
        sqp = ctx.enter_context(tc.tile_pool(name="sqp", bufs=4))
        up = ctx.enter_context(tc.tile_pool(name="up", bufs=3))
        apool = ctx.enter_context(tc.tile_pool(name="apool", bufs=12))
        rhp = ctx.enter_context(tc.tile_pool(name="rhp", bufs=5))
        bcp = ctx.enter_context(tc.tile_pool(name="bcp", bufs=5))
        smp = ctx.enter_context(tc.tile_pool(name="smp", bufs=2))
        pm = ctx.enter_context(tc.tile_pool(name="pm", bufs=3, space="PSUM"))
        pstat = ctx.enter_context(tc.tile_pool(name="pstat", bufs=2, space="PSUM"))

        def mm_block(W_sb, nkc, nmc, x, resid=None):
            """Paired outputs: (nmc // 2) PSUM tiles [128, 2R] (2 banks each)."""
            outs = []
            for pr in range(nmc // 2):
                ps = pm.tile([P, 2 * R], dt.float32, tag="mm")
                for half in range(2):
                    mc = pr * 2 + half
                    oap = ps[:, half * R:(half + 1) * R]
                    for kc in range(nkc):
                        nc.tensor.matmul(
                            oap, W_sb[:, ts(kc * nmc + mc, P)], x[:, ts(kc, R)],
                            start=(kc == 0),
                            stop=(kc == nkc - 1 and resid is None),
                        )
                    if resid is not None:
                        nc.tensor.matmul(oap, ident[:], resid[:, ts(mc, R)],
                                         start=False, stop=True)
                outs.append(ps)
            return outs

        def ln_a_multi(psls, q0_flag, s1_srcs, resid_dves):
            """Per-op interleaved across the G macrotiles."""
            n = len(psls)
            zs = [zp.tile([P, 4 * R], dt.bfloat16, name="z") for _ in range(n)]
            for pr in range(2):
                for g in range(n):
                    z, ps_list = zs[g], psls[g]
                    if q0_flag:
                        for half in range(2):
                            c = pr * 2 + half
                            nc.scalar.activation(
                                out=z[:, ts(c, R)],
                                in_=ps_list[pr][:, half * R:(half + 1) * R],
                                func=act_fn.Identity, bias=q0sb[:, c:c + 1])
                    elif resid_dves is not None:
                        sl = slice(pr * 2 * R, (pr + 1) * 2 * R)
                        nc.vector.tensor_tensor(out=z[:, sl], in0=ps_list[pr][:],
                                                in1=resid_dves[g][:, sl],
                                                op=alu.add)
                    else:
                        nc.scalar.copy(z[:, pr * 2 * R:(pr + 1) * 2 * R],
                                       ps_list[pr][:])
            s1ps = []
            for g in range(n):
                s1p = pstat.tile([1, R], dt.float32, tag="stat")
                if s1_srcs is not None:
                    rh, cs_sb = s1_srcs[g]
                    for kc in range(2):
                        nc.tensor.matmul(s1p[:, :], cs_sb[:, kc:kc + 1],
                                         rh[:, ts(kc, R)],
                                         start=(kc == 0), stop=(kc == 1))
                else:
                    for c in range(4):
                        nc.tensor.matmul(s1p[:, :], onesc[:], zs[g][:, ts(c, R)],
                                         start=(c == 0), stop=(c == 3))
                s1ps.append(s1p)
            sqs = [sqp.tile([P, 4 * R], dt.bfloat16, name="sq") for _ in range(n)]
            for g in range(n):
                nc.scalar.square(sqs[g][:, 0:2 * R], zs[g][:, 0:2 * R])
            for g in range(n):
                nc.vector.tensor_tensor(out=sqs[g][:, 2 * R:4 * R],
                                        in0=zs[g][:, 2 * R:4 * R],
                                        in1=zs[g][:, 2 * R:4 * R], op=alu.mult)
            s2ps = []
            for g in range(n):
                s2p = pstat.tile([1, R], dt.float32, tag="stat")
                for c in range(4):
                    nc.tensor.matmul(s2p[:, :], onesc[:], sqs[g][:, ts(c, R)],
                                     start=(c == 0), stop=(c == 3))
                s2ps.append(s2p)
            return [{"z": zs[g], "s1p": s1ps[g], "s2p": s2ps[g]}
                    for g in range(n)]

        def ln_b_multi(sts):
            n = len(sts)
            mus = [smp.tile([1, R], dt.bfloat16, tag="mu", name="mu") for _ in range(n)]
            for g in range(n):
                nc.scalar.mul(mus[g][:], sts[g]["s1p"][:], 1.0 / D)
            bpss = [pm.tile([P, 2 * R], dt.float32, tag="mm", name="bps") for _ in range(n)]
            for g in range(n):
                nc.tensor.matmul(bpss[g][:, 0:R], onesr[:], mus[g][:],
                                 start=True, stop=True)
            e2s = [smp.tile([1, R], dt.bfloat16, tag="e2", name="e2") for _ in range(n)]
            for g in range(n):
                nc.scalar.mul(e2s[g][:], sts[g]["s2p"][:], 1.0 / D)
            mu2s = [smp.tile([1, R], dt.bfloat16, tag="mu2", name="mu2") for _ in range(n)]
            for g in range(n):
                nc.vector.tensor_tensor(out=mu2s[g][:], in0=mus[g][:],
                                        in1=mus[g][:], op=alu.mult)
            vraws = [smp.tile([1, R], dt.bfloat16, tag="vraw", name="vraw") for _ in range(n)]
            for g in range(n):
                nc.vector.tensor_tensor(out=vraws[g][:], in0=e2s[g][:],
                                        in1=mu2s[g][:], op=alu.subtract)
            stds = [smp.tile([1, R], dt.float32, tag="std", name="std") for _ in range(n)]
            for g in range(n):
                nc.scalar.activation(out=stds[g][:], in_=vraws[g][:],
                                     func=act_fn.Sqrt, scale=1.0, bias=eps_t[:])
            rstdfs = [smp.tile([1, R], dt.float32, tag="rstdf", name="rstdf") for _ in range(n)]
            for g in range(n):
                nc.vector.reciprocal_approx_fast(out=rstdfs[g][:], in_=stds[g][:])
            rstds = [smp.tile([1, R], dt.bfloat16, tag="rstd", name="rstd") for _ in range(n)]
            for g in range(n):
                nc.scalar.copy(rstds[g][:], rstdfs[g][:])
            for g in range(n):
                nc.tensor.matmul(bpss[g][:, R:2 * R], onesr[:], rstds[g][:],
                                 start=True, stop=True)
            bcs = [bcp.tile([P, 2 * R], dt.bfloat16, tag="bc", name="bc") for _ in range(n)]
            for g in range(n):
                nc.scalar.copy(bcs[g][:], bpss[g][:])
            us = [up.tile([P, 4 * R], dt.bfloat16, name="u") for _ in range(n)]
            a_s = [apool.tile([P, 4 * R], dt.bfloat16, name="a") for _ in range(n)]
            for c in range(4):
                for g in range(n):
                    nc.vector.tensor_tensor(out=us[g][:, ts(c, R)],
                                            in0=sts[g]["z"][:, ts(c, R)],
                                            in1=bcs[g][:, 0:R], op=alu.subtract)
            for c in range(4):
                for g in range(n):
                    nc.vector.tensor_tensor(out=a_s[g][:, ts(c, R)],
                                            in0=us[g][:, ts(c, R)],
                                            in1=bcs[g][:, R:2 * R], op=alu.mult)
            return a_s

        def ffn_mm1(wsb_i, a_in):
            _, w1, _, _ = wsb_i
            hps = mm_block(w1, 4, 2, a_in)
            rh = rhp.tile([P, 2 * R], dt.bfloat16)
            nc.scalar.activation(out=rh[:], in_=hps[0][:], func=act_fn.Relu)
            return rh

        def head_mm(a2, a4):
            ps = pm.tile([P, 2 * R], dt.float32, tag="mm")
            for mc in range(2):
                oap = ps[:, mc * R:(mc + 1) * R]
                for kc in range(8):
                    x = a2 if kc < 4 else a4
                    nc.tensor.matmul(oap, h1sb[:, ts(kc * 2 + mc, P)],
                                     x[:, ts(kc % 4, R)],
                                     start=(kc == 0), stop=(kc == 7))
            rh = rhp.tile([P, 2 * R], dt.bfloat16)
            nc.scalar.activation(out=rh[:], in_=ps[:], func=act_fn.Relu)
            return rh

        def head_out(rh, t):
            lg = pstat.tile([1, R], dt.float32, tag="stat")
            for kc in range(2):
                nc.tensor.matmul(lg[:, :], h2sb[:, kc:kc + 1], rh[:, ts(kc, R)],
                                 start=(kc == 0), stop=(kc == 1))
            lgs = smp.tile([1, R], dt.float32, tag="lgs")
            nc.scalar.copy(lgs[:], lg[:])
            nc.sync.dma_start(logit_sb[t:t + 1, :], lgs[:])

        for t0 in range(0, NT, G):
            gs = list(range(G))
            tt = [t0 + g for g in gs]
            S = [dict() for _ in gs]

            for g in gs:
                cT = cin.tile([P, 4 * R], dt.bfloat16)
                nc.sync.dma_start(
                    cT[:], candT.ap()[tt[g] * P:(tt[g] + 1) * P, :])
                S[g]["c"] = cT
                S[g]["q"] = None

            for i in range(L):
                wa, w1, w2, csw2 = wsb[i]
                # q-stream attention
                for g in gs:
                    S[g]["ps"] = mm_block(wa, 4, 4, S[g]["c"])
                sts = ln_a_multi([S[g]["ps"] for g in gs], q0_flag=(i == 0),
                                 s1_srcs=None,
                                 resid_dves=None if i == 0 else
                                 [S[g]["q"] for g in gs])
                a1s = ln_b_multi(sts)
                # q-stream FFN
                rhs = [ffn_mm1(wsb[i], a1s[g]) for g in gs]
                for g in gs:
                    S[g]["ps"] = mm_block(w2, 2, 4, rhs[g])
                sts = ln_a_multi([S[g]["ps"] for g in gs], q0_flag=False,
                                 s1_srcs=[(rhs[g], csw2) for g in gs],
                                 resid_dves=a1s)
                a2s = ln_b_multi(sts)
                # c-stream attention
                for g in gs:
                    S[g]["ps"] = mm_block(wa, 4, 4, a2s[g])
                sts = ln_a_multi([S[g]["ps"] for g in gs], q0_flag=False,
                                 s1_srcs=None,
                                 resid_dves=[S[g]["c"] for g in gs])
                a3s = ln_b_multi(sts)
                # c-stream FFN
                rhs = [ffn_mm1(wsb[i], a3s[g]) for g in gs]
                for g in gs:
                    S[g]["ps"] = mm_block(w2, 2, 4, rhs[g])
                sts = ln_a_multi([S[g]["ps"] for g in gs], q0_flag=False,
                                 s1_srcs=[(rhs[g], csw2) for g in gs],
                                 resid_dves=a3s)
                a4s = ln_b_multi(sts)
                for g in gs:
                    S[g]["q"], S[g]["c"] = a2s[g], a4s[g]
                    S[g]["a2"], S[g]["a4"] = a2s[g], a4s[g]

            for g in gs:
                S[g]["hh"] = head_mm(S[g]["a2"], S[g]["a4"])
            for g in gs:
                head_out(S[g]["hh"], tt[g])

        fin = const.tile([NT, R], dt.float32, tag="fin")
        nc.scalar.activation(out=fin[:], in_=logit_sb[:], func=act_fn.Sigmoid)
        nc.sync.dma_start(scores.ap().rearrange("(t r) o -> t (r o)", r=R),
                          fin[:])

    nc.compile()
    return nc


def _get_program(rows_per_core: int):
    if rows_per_core not in _cache:
        _cache[rows_per_core] = _build_program(rows_per_core)
    return _cache[rows_per_core]


def _per_core_inputs(inputs) -> list:
    """Full inputs -> per-core input maps (shared weights + candT slice)."""
    arrs = _prep_host(inputs)
    cand = np.asarray(inputs["candidate_embeddings"]).astype(BF16)
    n = cand.shape[0]
    rows = n // NCORES
    in_maps = []
    for c in range(NCORES):
        m = dict(arrs)
        m["candT"] = _cand_T_for_core(cand[c * rows:(c + 1) * rows])
        in_maps.append(m)
    return in_maps


def kernel(**inputs) -> np.ndarray:
    from concourse.bass_utils import run_bass_kernel_spmd

    n = np.asarray(inputs["candidate_embeddings"]).shape[0]
    rows = n // NCORES
    nc = _get_program(rows)
    in_maps = _per_core_inputs(inputs)
    res = run_bass_kernel_spmd(nc, in_maps, list(range(NCORES)))
    out = np.concatenate([res.results[c]["scores"] for c in range(NCORES)], axis=0)
    return out.astype(np.float32)


if __name__ == "__main__":
    rows = int(sys.argv[1]) if len(sys.argv) > 1 else 4096
    nc = _build_program(rows)
    print("built ok:", rows)
